# revision 33
# baseline (speedup 1.0000x reference)
"""HGRN2Block kernel for 8 TRN2 NeuronCores.

Live path of the reference (the recurrence is dead code):
    x_proj = x @ W_proj + b_proj            # [B,L,3D]
    gate, _, ogate = split(x_proj, 3)       # middle third is DEAD
    out = (gate) * sigmoid(ogate)           # [B,L,D]
    out = out @ W_out + b_out               # [B,L,D]

Strategy:
  - Data-parallel over B*L rows: 16384 rows -> 2048 rows/core, no collectives.
  - Feature-major layout on device: host transposes x shard -> xT [D, rows]
    so every matmul contracts over the SBUF partition dim.
  - g-path and output matmuls in bf16 (PSUM accumulates fp32).
  - o-gate matmul in fp8e4m3 with DoubleRow (2 contraction rows per PE cell,
    ~1.7x bf16 MM rate). The sigmoid damps the fp8 quantization noise ~2.5x,
    so only this matmul tolerates fp8 within the error budget. Weights are
    pre-scaled x16 into fp8 range; the 1/16 folds into the activation scale.
  - Prologue is HBM-bandwidth-bound (~4.5MB needed in the first ~12us), so
    DRAM layouts are per-partition-contiguous (4-8KB DMA lines), transfers
    are chunked in need order, and spread across the gpsimd SWDGE queue
    (~210GB/s) and both HWDGE queues (~65GB/s each). The first row block
    runs all 8 o-gate PSUM groups before the first g-group so the PE only
    needs x8+wo8 (1MB) to start, and wg/wu are chunked by m-half to match
    their first-use times.
"""

import os

import numpy as np
import ml_dtypes

try:
    import concourse.bass as bass
except ImportError:
    import sys

    sys.path.insert(0, "/opt/trn_rl_repo")
    import concourse.bass as bass

import concourse.mybir as mybir
from concourse import bacc
from concourse.tile import TileContext
from concourse.bass_utils import run_bass_kernel_spmd

BF16 = ml_dtypes.bfloat16
F8 = ml_dtypes.float8_e4m3

B, L, D = 4, 4096, 1024
NCORES = 8
ROWS = B * L            # 16384
RPC = ROWS // NCORES    # 2048 rows per core
RB = 512                # moving free-dim per matmul (= one fp32 PSUM bank)
NRB = RPC // RB         # 4 row blocks per core
P = 128                 # SBUF partitions
KT = D // P             # 8 contraction tiles
KK = KT // 2            # 4 DoubleRow contraction pair-tiles
HD = D // 2             # weight m-half chunk
QD = D // 4             # weight m-quarter chunk
WO_SCALE = 16.0         # o-gate weights pre-scaled into fp8 range

_NC = None
LAST_RESULT = None      # BassKernelResults of the most recent run (for test.py)


def _build():
    nc = bacc.Bacc(trn_type="TRN2")
    f32 = mybir.dt.float32
    bf16 = mybir.dt.bfloat16
    fp8 = mybir.dt.float8e4
    DR = mybir.MatmulPerfMode.DoubleRow

    # all layouts per-partition contiguous for large DMA lines
    xb = nc.dram_tensor("xb", [NRB, P, KT, RB], bf16, kind="ExternalInput")
    x8 = nc.dram_tensor("x8", [NRB, P, KK, 2, RB], fp8, kind="ExternalInput")
    wg = nc.dram_tensor("wg", [4, P, KT, QD], bf16, kind="ExternalInput")
    wu = nc.dram_tensor("wu", [2, P, KT, HD], bf16, kind="ExternalInput")
    wo8 = nc.dram_tensor("wo8", [P, KK, 2, D], fp8, kind="ExternalInput")
    bias = nc.dram_tensor("bias", [P, 3, KT], f32, kind="ExternalInput")
    yT = nc.dram_tensor("yT", [D, RPC], bf16, kind="ExternalOutput")

    with TileContext(nc) as tc:
        with (
            tc.tile_pool(name="const", bufs=1) as cpool,
            tc.tile_pool(name="work", bufs=2) as wpool,
            tc.tile_pool(name="outp", bufs=4) as opool,
            tc.tile_pool(name="ps", bufs=2, space="PSUM") as pspool,
        ):
            biasS = cpool.tile([P, 3, KT], f32, tag="bias", name="biasS")
            wo8S = cpool.tile([P, KK, 2, D], fp8, tag="wo8", name="wo8S")
            x8S = cpool.tile([P, NRB, KK, 2, RB], fp8, tag="x8", name="x8S")
            xbS = cpool.tile([P, NRB, KT, RB], bf16, tag="xb", name="xbS")
            wgS = cpool.tile([P, KT, D], bf16, tag="wg", name="wgS")
            wuS = cpool.tile([P, KT, D], bf16, tag="wu", name="wuS")

            # --- Prologue DMAs, in need order per queue. Aggregate HBM read
            # BW in the prologue is ~310GB/s and the SDMA engines round-robin
            # between busy queues (~1/3 share each), so the critical set
            # (wo8 + x8 block 0, 1.5MB) is split across all three queues and
            # everything else follows in phase-A/B/C need order.
            # The SDMA engines give each busy queue an equal HBM share
            # (~100GB/s with three queues active), so inputs are balanced
            # ~3.5MB per queue and strictly need-ordered within each.
            # gpsimd SWDGE: o-gate weights (first half), g-path weights
            nc.gpsimd.dma_start(out=wo8S[:, 0:2], in_=wo8[:, 0:2])
            for q in range(4):
                nc.gpsimd.dma_start(out=wgS[:, :, q * QD : (q + 1) * QD], in_=wg[q])
            # sync HWDGE: x8 block 0, xb0 first quarter, remaining x8,
            # xb blocks 1-2; even-n yo output DMAs join in phase C.
            nc.sync.dma_start(out=x8S[:, 0], in_=x8[0])
            nc.sync.dma_start(out=xbS[:, 0, 0:2], in_=xb[0, :, 0:2])
            for rb in range(1, NRB):
                nc.sync.dma_start(out=x8S[:, rb], in_=x8[rb])
            nc.sync.dma_start(out=xbS[:, 1], in_=xb[1])
            nc.sync.dma_start(out=xbS[:, 2], in_=xb[2])
            # scalar HWDGE: bias, o-gate weights kk23, xb0 quarters,
            # wu, xb block 3; odd-n yo output DMAs join in phase C.
            nc.scalar.dma_start(out=biasS, in_=bias[:])
            nc.scalar.dma_start(out=wo8S[:, 2:4], in_=wo8[:, 2:4])
            nc.scalar.dma_start(out=xbS[:, 0, 2:4], in_=xb[0, :, 2:4])
            nc.scalar.dma_start(out=xbS[:, 0, 4:6], in_=xb[0, :, 4:6])
            nc.scalar.dma_start(out=xbS[:, 0, 6:8], in_=xb[0, :, 6:8])
            nc.scalar.dma_start(out=wuS[:, :, 0:HD], in_=wu[0])
            nc.scalar.dma_start(out=xbS[:, 3], in_=xb[3])
            nc.scalar.dma_start(out=wuS[:, :, HD:D], in_=wu[1])

            # Warm-up: HAM ungates the PE clock after ~3.4us of sustained
            # activity. Spin until the critical DMA set (wo8 + first x8
            # blocks, ~19us in) has fully landed, so the real matmul stream
            # starts warm and runs dense -- a ragged early start costs more
            # in HAM re-throttle (cold runs at half rate) than it gains.
            wz = cpool.tile([P, RB], bf16, tag="wz", name="wz")
            nc.vector.memset(wz, 0.0)
            spin = pspool.tile([P, RB], f32, tag="py", name="spin", bufs=2)
            for _ in range(42):
                nc.tensor.matmul(spin, lhsT=wz[:, :P], rhs=wz, start=True, stop=True)

            # ---- phase A: o-gate for all row blocks (needs only wo8 + x8)
            sig_t = {}
            for rb in range(NRB):
                for m in range(KT):
                    msl = slice(m * P, (m + 1) * P)
                    po = pspool.tile([P, RB], f32, tag="po", name=f"po{rb}_{m}", bufs=4)
                    for kk in range(KK):
                        nc.tensor.matmul(
                            po,
                            lhsT=wo8S[:, kk, :, msl],
                            rhs=x8S[:, rb, kk],
                            start=(kk == 0),
                            stop=(kk == KK - 1),
                            perf_mode=DR,
                        )
                    sig = opool.tile(
                        [P, RB], bf16, tag=f"sig{rb}_{m}", name=f"sig{rb}_{m}",
                        bufs=1,
                    )
                    nc.scalar.activation(
                        out=sig, in_=po,
                        func=mybir.ActivationFunctionType.Sigmoid,
                        bias=biasS[:, 1, m : m + 1], scale=1.0 / WO_SCALE,
                    )
                    sig_t[rb, m] = sig

            # ---- phase B: g-path for all row blocks: gT = (hT+bg) * sig
            gS = {}
            for rb in range(NRB):
                for m in range(KT):
                    msl = slice(m * P, (m + 1) * P)
                    ph = pspool.tile([P, RB], f32, tag="ph", name=f"ph{rb}_{m}", bufs=2)
                    for k in range(KT):
                        nc.tensor.matmul(
                            ph, lhsT=wgS[:, k, msl], rhs=xbS[:, rb, k],
                            start=(k == 0), stop=(k == KT - 1),
                        )
                    g = wpool.tile(
                        [P, RB], bf16, tag=f"g{rb}_{m}", name=f"g{rb}_{m}", bufs=1
                    )
                    nc.vector.scalar_tensor_tensor(
                        out=g, in0=ph, scalar=biasS[:, 0, m : m + 1],
                        in1=sig_t[rb, m],
                        op0=mybir.AluOpType.add, op1=mybir.AluOpType.mult,
                    )
                    gS[rb, m] = g

            # ---- phase C: layer 2 for all row blocks (+ b_out)
            for rb in range(NRB):
                cols = slice(rb * RB, (rb + 1) * RB)
                for n in range(KT):
                    nsl = slice(n * P, (n + 1) * P)
                    py = pspool.tile([P, RB], f32, tag="py", name=f"py{rb}_{n}", bufs=2)
                    for m in range(KT):
                        nc.tensor.matmul(
                            py, lhsT=wuS[:, m, nsl], rhs=gS[rb, m],
                            start=(m == 0), stop=(m == KT - 1),
                        )
                    yo = opool.tile([P, RB], bf16, tag="yo", name=f"yo{rb}_{n}")
                    # bias-add on DVE, keeping ScalarE free for sigmoids
                    nc.vector.tensor_scalar_add(yo, py, biasS[:, 2, n : n + 1])
                    # outputs split across both HWDGE queues (idle by phase C)
                    # so the drain keeps ahead of the production rate
                    if n % 2 == 0:
                        nc.sync.dma_start(out=yT[nsl, cols], in_=yo)
                    else:
                        nc.scalar.dma_start(out=yT[nsl, cols], in_=yo)
    nc.finalize()
    return nc


def kernel(x, W_proj, b_proj, W_out, b_out, layer_idx=0, num_layers=12):
    global _NC, LAST_RESULT
    x = np.asarray(x, dtype=np.float32)
    W_proj = np.asarray(W_proj, dtype=np.float32)
    b_proj = np.asarray(b_proj, dtype=np.float32)
    W_out = np.asarray(W_out, dtype=np.float32)
    b_out = np.asarray(b_out, dtype=np.float32)

    wg_f = W_proj[:, :D]
    wo_f = W_proj[:, 2 * D : 3 * D]
    # [4, P, KT, QD]: m-quarter-major, per-partition contiguous
    wg_h = np.ascontiguousarray(
        wg_f.reshape(KT, P, 4, QD).transpose(2, 1, 0, 3)
    ).astype(BF16)
    wu_h = np.ascontiguousarray(
        W_out.reshape(KT, P, 2, HD).transpose(2, 1, 0, 3)
    ).astype(BF16)
    # [P, KK, 2, D]
    wo8_h = np.ascontiguousarray(
        (wo_f * WO_SCALE).reshape(KK, 2, P, D).transpose(2, 0, 1, 3)
    ).astype(F8)
    bias_h = np.ascontiguousarray(
        np.stack(
            [
                b_proj[:D].reshape(KT, P).T,
                b_proj[2 * D : 3 * D].reshape(KT, P).T,
                b_out.reshape(KT, P).T,
            ],
            axis=1,
        )
    )  # [P, 3, KT]

    xf = x.reshape(ROWS, D)
    in_maps = []
    for c in range(NCORES):
        xT = np.ascontiguousarray(xf[c * RPC : (c + 1) * RPC, :].T)  # [D, RPC]
        # [NRB, P, KT, RB]
        xb_h = np.ascontiguousarray(
            xT.reshape(KT, P, NRB, RB).transpose(2, 1, 0, 3)
        ).astype(BF16)
        # [NRB, P, KK, 2, RB]: block-major fp8 pairs for DoubleRow
        x8_h = np.ascontiguousarray(
            xT.reshape(KK, 2, P, NRB, RB).transpose(3, 2, 0, 1, 4)
        ).astype(F8)
        in_maps.append(
            {"xb": xb_h, "x8": x8_h, "wg": wg_h, "wu": wu_h,
             "wo8": wo8_h, "bias": bias_h}
        )

    if _NC is None:
        _NC = _build()

    trace = os.environ.get("HGRN_TRACE", "0") == "1"
    LAST_RESULT = run_bass_kernel_spmd(
        _NC, in_maps, core_ids=list(range(NCORES)), trace=trace,
        tmpdir=os.environ.get("HGRN_TMPDIR"),
    )
    y = np.empty((ROWS, D), dtype=np.float32)
    for c in range(NCORES):
        y[c * RPC : (c + 1) * RPC, :] = np.asarray(
            LAST_RESULT.results[c]["yT"], dtype=np.float32
        ).T
    return y.reshape(B, L, D)


# revision 35
# speedup vs baseline: 1.0544x; 1.0544x over previous
"""HGRN2Block kernel for 8 TRN2 NeuronCores.

Live path of the reference (the recurrence is dead code):
    x_proj = x @ W_proj + b_proj            # [B,L,3D]
    gate, _, ogate = split(x_proj, 3)       # middle third is DEAD
    out = (gate) * sigmoid(ogate)           # [B,L,D]
    out = out @ W_out + b_out               # [B,L,D]

Strategy:
  - Data-parallel over B*L rows: 16384 rows -> 2048 rows/core, no collectives.
  - Feature-major layout on device: host transposes x shard -> xT [D, rows]
    so every matmul contracts over the SBUF partition dim.
  - g-path and output matmuls in bf16 (PSUM accumulates fp32).
  - o-gate matmul in fp8e4m3 with DoubleRow (2 contraction rows per PE cell,
    ~1.7x bf16 MM rate). The sigmoid damps the fp8 quantization noise ~2.5x,
    so only this matmul tolerates fp8 within the error budget. Weights are
    pre-scaled x16 into fp8 range; the 1/16 folds into the activation scale.
  - Prologue is HBM-bandwidth-bound (~4.5MB needed in the first ~12us), so
    DRAM layouts are per-partition-contiguous (4-8KB DMA lines), transfers
    are chunked in need order, and spread across the gpsimd SWDGE queue
    (~210GB/s) and both HWDGE queues (~65GB/s each). The first row block
    runs all 8 o-gate PSUM groups before the first g-group so the PE only
    needs x8+wo8 (1MB) to start, and wg/wu are chunked by m-half to match
    their first-use times.
"""

import os

import numpy as np
import ml_dtypes

try:
    import concourse.bass as bass
except ImportError:
    import sys

    sys.path.insert(0, "/opt/trn_rl_repo")
    import concourse.bass as bass

import concourse.mybir as mybir
from concourse import bacc
from concourse.tile import TileContext
from concourse.bass_utils import run_bass_kernel_spmd

BF16 = ml_dtypes.bfloat16
F8 = ml_dtypes.float8_e4m3

B, L, D = 4, 4096, 1024
NCORES = 8
ROWS = B * L            # 16384
RPC = ROWS // NCORES    # 2048 rows per core
RB = 512                # moving free-dim per matmul (= one fp32 PSUM bank)
NRB = RPC // RB         # 4 row blocks per core
P = 128                 # SBUF partitions
KT = D // P             # 8 contraction tiles
KK = KT // 2            # 4 DoubleRow contraction pair-tiles
HD = D // 2             # weight m-half chunk
QD = D // 4             # weight m-quarter chunk
WO_SCALE = 16.0         # o-gate weights pre-scaled into fp8 range

_NC = None
LAST_RESULT = None      # BassKernelResults of the most recent run (for test.py)


def _build():
    nc = bacc.Bacc(trn_type="TRN2")
    f32 = mybir.dt.float32
    bf16 = mybir.dt.bfloat16
    fp8 = mybir.dt.float8e4
    DR = mybir.MatmulPerfMode.DoubleRow

    # all layouts per-partition contiguous for large DMA lines
    xb = nc.dram_tensor("xb", [NRB, P, KT, RB], bf16, kind="ExternalInput")
    x8 = nc.dram_tensor("x8", [NRB, P, KK, 2, RB], fp8, kind="ExternalInput")
    wg = nc.dram_tensor("wg", [4, P, KT, QD], bf16, kind="ExternalInput")
    wu = nc.dram_tensor("wu", [2, P, KT, HD], bf16, kind="ExternalInput")
    wo8 = nc.dram_tensor("wo8", [P, KK, 2, D], fp8, kind="ExternalInput")
    bias = nc.dram_tensor("bias", [P, 3, KT], f32, kind="ExternalInput")
    yT = nc.dram_tensor("yT", [D, RPC], bf16, kind="ExternalOutput")

    with TileContext(nc) as tc:
        with (
            tc.tile_pool(name="const", bufs=1) as cpool,
            tc.tile_pool(name="work", bufs=2) as wpool,
            tc.tile_pool(name="outp", bufs=4) as opool,
            tc.tile_pool(name="ps", bufs=2, space="PSUM") as pspool,
        ):
            biasS = cpool.tile([P, 3, KT], f32, tag="bias", name="biasS")
            wo8S = cpool.tile([P, KK, 2, D], fp8, tag="wo8", name="wo8S")
            x8S = cpool.tile([P, NRB, KK, 2, RB], fp8, tag="x8", name="x8S")
            xbS = cpool.tile([P, NRB, KT, RB], bf16, tag="xb", name="xbS")
            wgS = cpool.tile([P, KT, D], bf16, tag="wg", name="wgS")
            wuS = cpool.tile([P, KT, D], bf16, tag="wu", name="wuS")

            # --- Prologue DMAs, in need order per queue. Aggregate HBM read
            # BW in the prologue is ~310GB/s and the SDMA engines round-robin
            # between busy queues (~1/3 share each), so the critical set
            # (wo8 + x8 block 0, 1.5MB) is split across all three queues and
            # everything else follows in phase-A/B/C need order.
            # The SDMA engines give each busy queue an equal HBM share
            # (~100GB/s with three queues active), so inputs are balanced
            # ~3.5MB per queue and strictly need-ordered within each.
            # gpsimd SWDGE: o-gate weights, g-path weights
            nc.gpsimd.dma_start(out=wo8S[:, 0:2], in_=wo8[:, 0:2])
            nc.gpsimd.dma_start(out=wo8S[:, 2:4], in_=wo8[:, 2:4])
            for q in range(4):
                nc.gpsimd.dma_start(out=wgS[:, :, q * QD : (q + 1) * QD], in_=wg[q])
            # sync HWDGE: x8 blocks in phase-A order, xb blocks 1-2;
            # even-n yo output DMAs join this queue in phase C.
            for rb in range(NRB):
                nc.sync.dma_start(out=x8S[:, rb], in_=x8[rb])
            nc.sync.dma_start(out=xbS[:, 1], in_=xb[1])
            nc.sync.dma_start(out=xbS[:, 2], in_=xb[2])
            # scalar HWDGE: bias, xb block 0 halves, wu, xb block 3;
            # odd-n yo output DMAs join this queue in phase C.
            nc.scalar.dma_start(out=biasS, in_=bias[:])
            nc.scalar.dma_start(out=xbS[:, 0, 4:8], in_=xb[0, :, 4:8])
            nc.scalar.dma_start(out=xbS[:, 0, 0:4], in_=xb[0, :, 0:4])
            nc.scalar.dma_start(out=wuS[:, :, 0:HD], in_=wu[0])
            nc.scalar.dma_start(out=xbS[:, 3], in_=xb[3])
            nc.scalar.dma_start(out=wuS[:, :, HD:D], in_=wu[1])

            # Warm-up: HAM ungates the PE clock after ~3.4us of sustained
            # activity. Spin until the critical DMA set (wo8 + first x8
            # blocks, ~19us in) has fully landed, so the real matmul stream
            # starts warm and runs dense -- a ragged early start costs more
            # in HAM re-throttle (cold runs at half rate) than it gains.
            wz = cpool.tile([P, RB], bf16, tag="wz", name="wz")
            nc.vector.memset(wz, 0.0)
            spin = pspool.tile([P, RB], f32, tag="py", name="spin", bufs=2)
            for _ in range(46):
                nc.tensor.matmul(spin, lhsT=wz[:, :P], rhs=wz, start=True, stop=True)

            # ---- phase A: o-gate for all row blocks (needs only wo8 + x8)
            sig_t = {}
            for rb in range(NRB):
                for m in range(KT):
                    msl = slice(m * P, (m + 1) * P)
                    po = pspool.tile([P, RB], f32, tag="po", name=f"po{rb}_{m}", bufs=4)
                    for kk in range(KK):
                        nc.tensor.matmul(
                            po,
                            lhsT=wo8S[:, kk, :, msl],
                            rhs=x8S[:, rb, kk],
                            start=(kk == 0),
                            stop=(kk == KK - 1),
                            perf_mode=DR,
                        )
                    sig = opool.tile(
                        [P, RB], bf16, tag=f"sig{rb}_{m}", name=f"sig{rb}_{m}",
                        bufs=1,
                    )
                    nc.scalar.activation(
                        out=sig, in_=po,
                        func=mybir.ActivationFunctionType.Sigmoid,
                        bias=biasS[:, 1, m : m + 1], scale=1.0 / WO_SCALE,
                    )
                    sig_t[rb, m] = sig

            # ---- phase B: g-path for all row blocks: gT = (hT+bg) * sig
            gS = {}
            for rb in range(NRB):
                for m in range(KT):
                    msl = slice(m * P, (m + 1) * P)
                    ph = pspool.tile([P, RB], f32, tag="ph", name=f"ph{rb}_{m}", bufs=2)
                    for k in range(KT):
                        nc.tensor.matmul(
                            ph, lhsT=wgS[:, k, msl], rhs=xbS[:, rb, k],
                            start=(k == 0), stop=(k == KT - 1),
                        )
                    g = wpool.tile(
                        [P, RB], bf16, tag=f"g{rb}_{m}", name=f"g{rb}_{m}", bufs=1
                    )
                    nc.vector.scalar_tensor_tensor(
                        out=g, in0=ph, scalar=biasS[:, 0, m : m + 1],
                        in1=sig_t[rb, m],
                        op0=mybir.AluOpType.add, op1=mybir.AluOpType.mult,
                    )
                    gS[rb, m] = g

            # ---- phase C: layer 2 for all row blocks (+ b_out)
            for rb in range(NRB):
                cols = slice(rb * RB, (rb + 1) * RB)
                for n in range(KT):
                    nsl = slice(n * P, (n + 1) * P)
                    py = pspool.tile([P, RB], f32, tag="py", name=f"py{rb}_{n}", bufs=2)
                    for m in range(KT):
                        nc.tensor.matmul(
                            py, lhsT=wuS[:, m, nsl], rhs=gS[rb, m],
                            start=(m == 0), stop=(m == KT - 1),
                        )
                    yo = opool.tile([P, RB], bf16, tag="yo", name=f"yo{rb}_{n}")
                    # bias-add on DVE, keeping ScalarE free for sigmoids
                    nc.vector.tensor_scalar_add(yo, py, biasS[:, 2, n : n + 1])
                    # outputs split across both HWDGE queues (idle by phase C)
                    # so the drain keeps ahead of the production rate
                    if n % 2 == 0:
                        nc.sync.dma_start(out=yT[nsl, cols], in_=yo)
                    else:
                        nc.scalar.dma_start(out=yT[nsl, cols], in_=yo)
    nc.finalize()
    return nc


def kernel(x, W_proj, b_proj, W_out, b_out, layer_idx=0, num_layers=12):
    global _NC, LAST_RESULT
    x = np.asarray(x, dtype=np.float32)
    W_proj = np.asarray(W_proj, dtype=np.float32)
    b_proj = np.asarray(b_proj, dtype=np.float32)
    W_out = np.asarray(W_out, dtype=np.float32)
    b_out = np.asarray(b_out, dtype=np.float32)

    wg_f = W_proj[:, :D]
    wo_f = W_proj[:, 2 * D : 3 * D]
    # [4, P, KT, QD]: m-quarter-major, per-partition contiguous
    wg_h = np.ascontiguousarray(
        wg_f.reshape(KT, P, 4, QD).transpose(2, 1, 0, 3)
    ).astype(BF16)
    wu_h = np.ascontiguousarray(
        W_out.reshape(KT, P, 2, HD).transpose(2, 1, 0, 3)
    ).astype(BF16)
    # [P, KK, 2, D]
    wo8_h = np.ascontiguousarray(
        (wo_f * WO_SCALE).reshape(KK, 2, P, D).transpose(2, 0, 1, 3)
    ).astype(F8)
    bias_h = np.ascontiguousarray(
        np.stack(
            [
                b_proj[:D].reshape(KT, P).T,
                b_proj[2 * D : 3 * D].reshape(KT, P).T,
                b_out.reshape(KT, P).T,
            ],
            axis=1,
        )
    )  # [P, 3, KT]

    xf = x.reshape(ROWS, D)
    in_maps = []
    for c in range(NCORES):
        xT = np.ascontiguousarray(xf[c * RPC : (c + 1) * RPC, :].T)  # [D, RPC]
        # [NRB, P, KT, RB]
        xb_h = np.ascontiguousarray(
            xT.reshape(KT, P, NRB, RB).transpose(2, 1, 0, 3)
        ).astype(BF16)
        # [NRB, P, KK, 2, RB]: block-major fp8 pairs for DoubleRow
        x8_h = np.ascontiguousarray(
            xT.reshape(KK, 2, P, NRB, RB).transpose(3, 2, 0, 1, 4)
        ).astype(F8)
        in_maps.append(
            {"xb": xb_h, "x8": x8_h, "wg": wg_h, "wu": wu_h,
             "wo8": wo8_h, "bias": bias_h}
        )

    if _NC is None:
        _NC = _build()

    trace = os.environ.get("HGRN_TRACE", "0") == "1"
    LAST_RESULT = run_bass_kernel_spmd(
        _NC, in_maps, core_ids=list(range(NCORES)), trace=trace,
        tmpdir=os.environ.get("HGRN_TMPDIR"),
    )
    y = np.empty((ROWS, D), dtype=np.float32)
    for c in range(NCORES):
        y[c * RPC : (c + 1) * RPC, :] = np.asarray(
            LAST_RESULT.results[c]["yT"], dtype=np.float32
        ).T
    return y.reshape(B, L, D)


# revision 36
# speedup vs baseline: 1.0693x; 1.0142x over previous
"""HGRN2Block kernel for 8 TRN2 NeuronCores.

Live path of the reference (the recurrence is dead code):
    x_proj = x @ W_proj + b_proj            # [B,L,3D]
    gate, _, ogate = split(x_proj, 3)       # middle third is DEAD
    out = (gate) * sigmoid(ogate)           # [B,L,D]
    out = out @ W_out + b_out               # [B,L,D]

Strategy:
  - Data-parallel over B*L rows: 16384 rows -> 2048 rows/core, no collectives.
  - Feature-major layout on device: host transposes x shard -> xT [D, rows]
    so every matmul contracts over the SBUF partition dim.
  - g-path and output matmuls in bf16 (PSUM accumulates fp32).
  - o-gate matmul in fp8e4m3 with DoubleRow (2 contraction rows per PE cell,
    ~1.7x bf16 MM rate). The sigmoid damps the fp8 quantization noise ~2.5x,
    so only this matmul tolerates fp8 within the error budget. Weights are
    pre-scaled x16 into fp8 range; the 1/16 folds into the activation scale.
  - Prologue is HBM-bandwidth-bound (~4.5MB needed in the first ~12us), so
    DRAM layouts are per-partition-contiguous (4-8KB DMA lines), transfers
    are chunked in need order, and spread across the gpsimd SWDGE queue
    (~210GB/s) and both HWDGE queues (~65GB/s each). The first row block
    runs all 8 o-gate PSUM groups before the first g-group so the PE only
    needs x8+wo8 (1MB) to start, and wg/wu are chunked by m-half to match
    their first-use times.
"""

import os

import numpy as np
import ml_dtypes

try:
    import concourse.bass as bass
except ImportError:
    import sys

    sys.path.insert(0, "/opt/trn_rl_repo")
    import concourse.bass as bass

import concourse.mybir as mybir
from concourse import bacc
from concourse.tile import TileContext
from concourse.bass_utils import run_bass_kernel_spmd

BF16 = ml_dtypes.bfloat16
F8 = ml_dtypes.float8_e4m3

B, L, D = 4, 4096, 1024
NCORES = 8
ROWS = B * L            # 16384
RPC = ROWS // NCORES    # 2048 rows per core
RB = 512                # moving free-dim per matmul (= one fp32 PSUM bank)
NRB = RPC // RB         # 4 row blocks per core
P = 128                 # SBUF partitions
KT = D // P             # 8 contraction tiles
KK = KT // 2            # 4 DoubleRow contraction pair-tiles
HD = D // 2             # weight m-half chunk
QD = D // 4             # weight m-quarter chunk
WO_SCALE = 16.0         # o-gate weights pre-scaled into fp8 range

_NC = None
LAST_RESULT = None      # BassKernelResults of the most recent run (for test.py)


def _build():
    nc = bacc.Bacc(trn_type="TRN2")
    f32 = mybir.dt.float32
    bf16 = mybir.dt.bfloat16
    fp8 = mybir.dt.float8e4
    DR = mybir.MatmulPerfMode.DoubleRow

    # all layouts per-partition contiguous for large DMA lines
    xb = nc.dram_tensor("xb", [NRB, P, KT, RB], bf16, kind="ExternalInput")
    x8 = nc.dram_tensor("x8", [NRB, P, KK, 2, RB], fp8, kind="ExternalInput")
    wg = nc.dram_tensor("wg", [4, P, KT, QD], bf16, kind="ExternalInput")
    wu = nc.dram_tensor("wu", [2, P, KT, HD], bf16, kind="ExternalInput")
    wo8 = nc.dram_tensor("wo8", [P, KK, 2, D], fp8, kind="ExternalInput")
    bias = nc.dram_tensor("bias", [P, 3, KT], f32, kind="ExternalInput")
    yT = nc.dram_tensor("yT", [D, RPC], bf16, kind="ExternalOutput")

    with TileContext(nc) as tc:
        with (
            tc.tile_pool(name="const", bufs=1) as cpool,
            tc.tile_pool(name="work", bufs=2) as wpool,
            tc.tile_pool(name="outp", bufs=4) as opool,
            tc.tile_pool(name="ps", bufs=2, space="PSUM") as pspool,
        ):
            biasS = cpool.tile([P, 3, KT], f32, tag="bias", name="biasS")
            wo8S = cpool.tile([P, KK, 2, D], fp8, tag="wo8", name="wo8S")
            x8S = cpool.tile([P, NRB, KK, 2, RB], fp8, tag="x8", name="x8S")
            xbS = cpool.tile([P, NRB, KT, RB], bf16, tag="xb", name="xbS")
            wgS = cpool.tile([P, KT, D], bf16, tag="wg", name="wgS")
            wuS = cpool.tile([P, KT, D], bf16, tag="wu", name="wuS")

            # --- Prologue DMAs, in need order per queue. Aggregate HBM read
            # BW in the prologue is ~310GB/s and the SDMA engines round-robin
            # between busy queues (~1/3 share each), so the critical set
            # (wo8 + x8 block 0, 1.5MB) is split across all three queues and
            # everything else follows in phase-A/B/C need order.
            # The SDMA engines give each busy queue an equal HBM share
            # (~100GB/s with three queues active), so inputs are balanced
            # ~3.5MB per queue and strictly need-ordered within each.
            # gpsimd SWDGE: o-gate weights, g-path weights
            nc.gpsimd.dma_start(out=wo8S[:, 0:2], in_=wo8[:, 0:2])
            nc.gpsimd.dma_start(out=wo8S[:, 2:4], in_=wo8[:, 2:4])
            for q in range(4):
                nc.gpsimd.dma_start(out=wgS[:, :, q * QD : (q + 1) * QD], in_=wg[q])
            # sync HWDGE: x8 blocks in phase-A order, xb blocks 1-2;
            # even-n yo output DMAs join this queue in phase C.
            for rb in range(NRB):
                nc.sync.dma_start(out=x8S[:, rb], in_=x8[rb])
            nc.sync.dma_start(out=xbS[:, 1], in_=xb[1])
            nc.sync.dma_start(out=xbS[:, 2], in_=xb[2])
            # scalar HWDGE: bias, xb block 0 halves, wu, xb block 3;
            # odd-n yo output DMAs join this queue in phase C.
            nc.scalar.dma_start(out=biasS, in_=bias[:])
            nc.scalar.dma_start(out=xbS[:, 0, 4:8], in_=xb[0, :, 4:8])
            nc.scalar.dma_start(out=xbS[:, 0, 0:4], in_=xb[0, :, 0:4])
            nc.scalar.dma_start(out=wuS[:, :, 0:HD], in_=wu[0])
            nc.scalar.dma_start(out=xbS[:, 3], in_=xb[3])
            nc.scalar.dma_start(out=wuS[:, :, HD:D], in_=wu[1])

            # Warm-up: HAM ungates the PE clock after ~3.4us of sustained
            # activity. Spin until the critical DMA set (wo8 + first x8
            # blocks, ~19us in) has fully landed, so the real matmul stream
            # starts warm and runs dense -- a ragged early start costs more
            # in HAM re-throttle (cold runs at half rate) than it gains.
            wz = cpool.tile([P, RB], bf16, tag="wz", name="wz")
            nc.vector.memset(wz, 0.0)
            spin = pspool.tile([P, RB], f32, tag="ps", name="spin", bufs=8)
            for _ in range(46):
                nc.tensor.matmul(spin, lhsT=wz[:, :P], rhs=wz, start=True, stop=True)

            # ---- phase A: o-gate for all row blocks (needs only wo8 + x8)
            sig_t = {}
            for rb in range(NRB):
                for m in range(KT):
                    msl = slice(m * P, (m + 1) * P)
                    po = pspool.tile([P, RB], f32, tag="ps", name=f"po{rb}_{m}", bufs=8)
                    for kk in range(KK):
                        nc.tensor.matmul(
                            po,
                            lhsT=wo8S[:, kk, :, msl],
                            rhs=x8S[:, rb, kk],
                            start=(kk == 0),
                            stop=(kk == KK - 1),
                            perf_mode=DR,
                        )
                    sig = opool.tile(
                        [P, RB], bf16, tag=f"sig{rb}_{m}", name=f"sig{rb}_{m}",
                        bufs=1,
                    )
                    nc.scalar.activation(
                        out=sig, in_=po,
                        func=mybir.ActivationFunctionType.Sigmoid,
                        bias=biasS[:, 1, m : m + 1], scale=1.0 / WO_SCALE,
                    )
                    sig_t[rb, m] = sig

            # ---- phase B: g-path for all row blocks: gT = (hT+bg) * sig
            gS = {}
            for rb in range(NRB):
                for m in range(KT):
                    msl = slice(m * P, (m + 1) * P)
                    ph = pspool.tile([P, RB], f32, tag="ps", name=f"ph{rb}_{m}", bufs=8)
                    for k in range(KT):
                        nc.tensor.matmul(
                            ph, lhsT=wgS[:, k, msl], rhs=xbS[:, rb, k],
                            start=(k == 0), stop=(k == KT - 1),
                        )
                    g = wpool.tile(
                        [P, RB], bf16, tag=f"g{rb}_{m}", name=f"g{rb}_{m}", bufs=1
                    )
                    nc.vector.scalar_tensor_tensor(
                        out=g, in0=ph, scalar=biasS[:, 0, m : m + 1],
                        in1=sig_t[rb, m],
                        op0=mybir.AluOpType.add, op1=mybir.AluOpType.mult,
                    )
                    gS[rb, m] = g

            # ---- phase C: layer 2 for all row blocks (+ b_out)
            for rb in range(NRB):
                cols = slice(rb * RB, (rb + 1) * RB)
                for n in range(KT):
                    nsl = slice(n * P, (n + 1) * P)
                    py = pspool.tile([P, RB], f32, tag="ps", name=f"py{rb}_{n}", bufs=8)
                    for m in range(KT):
                        nc.tensor.matmul(
                            py, lhsT=wuS[:, m, nsl], rhs=gS[rb, m],
                            start=(m == 0), stop=(m == KT - 1),
                        )
                    yo = opool.tile([P, RB], bf16, tag="yo", name=f"yo{rb}_{n}")
                    # bias-add on DVE, keeping ScalarE free for sigmoids
                    nc.vector.tensor_scalar_add(yo, py, biasS[:, 2, n : n + 1])
                    # outputs split across both HWDGE queues (idle by phase C)
                    # so the drain keeps ahead of the production rate
                    if n % 2 == 0:
                        nc.sync.dma_start(out=yT[nsl, cols], in_=yo)
                    else:
                        nc.scalar.dma_start(out=yT[nsl, cols], in_=yo)
    nc.finalize()
    return nc


def kernel(x, W_proj, b_proj, W_out, b_out, layer_idx=0, num_layers=12):
    global _NC, LAST_RESULT
    x = np.asarray(x, dtype=np.float32)
    W_proj = np.asarray(W_proj, dtype=np.float32)
    b_proj = np.asarray(b_proj, dtype=np.float32)
    W_out = np.asarray(W_out, dtype=np.float32)
    b_out = np.asarray(b_out, dtype=np.float32)

    wg_f = W_proj[:, :D]
    wo_f = W_proj[:, 2 * D : 3 * D]
    # [4, P, KT, QD]: m-quarter-major, per-partition contiguous
    wg_h = np.ascontiguousarray(
        wg_f.reshape(KT, P, 4, QD).transpose(2, 1, 0, 3)
    ).astype(BF16)
    wu_h = np.ascontiguousarray(
        W_out.reshape(KT, P, 2, HD).transpose(2, 1, 0, 3)
    ).astype(BF16)
    # [P, KK, 2, D]
    wo8_h = np.ascontiguousarray(
        (wo_f * WO_SCALE).reshape(KK, 2, P, D).transpose(2, 0, 1, 3)
    ).astype(F8)
    bias_h = np.ascontiguousarray(
        np.stack(
            [
                b_proj[:D].reshape(KT, P).T,
                b_proj[2 * D : 3 * D].reshape(KT, P).T,
                b_out.reshape(KT, P).T,
            ],
            axis=1,
        )
    )  # [P, 3, KT]

    xf = x.reshape(ROWS, D)
    in_maps = []
    for c in range(NCORES):
        xT = np.ascontiguousarray(xf[c * RPC : (c + 1) * RPC, :].T)  # [D, RPC]
        # [NRB, P, KT, RB]
        xb_h = np.ascontiguousarray(
            xT.reshape(KT, P, NRB, RB).transpose(2, 1, 0, 3)
        ).astype(BF16)
        # [NRB, P, KK, 2, RB]: block-major fp8 pairs for DoubleRow
        x8_h = np.ascontiguousarray(
            xT.reshape(KK, 2, P, NRB, RB).transpose(3, 2, 0, 1, 4)
        ).astype(F8)
        in_maps.append(
            {"xb": xb_h, "x8": x8_h, "wg": wg_h, "wu": wu_h,
             "wo8": wo8_h, "bias": bias_h}
        )

    if _NC is None:
        _NC = _build()

    trace = os.environ.get("HGRN_TRACE", "0") == "1"
    LAST_RESULT = run_bass_kernel_spmd(
        _NC, in_maps, core_ids=list(range(NCORES)), trace=trace,
        tmpdir=os.environ.get("HGRN_TMPDIR"),
    )
    y = np.empty((ROWS, D), dtype=np.float32)
    for c in range(NCORES):
        y[c * RPC : (c + 1) * RPC, :] = np.asarray(
            LAST_RESULT.results[c]["yT"], dtype=np.float32
        ).T
    return y.reshape(B, L, D)


# revision 38
# speedup vs baseline: 1.0709x; 1.0015x over previous
"""HGRN2Block kernel for 8 TRN2 NeuronCores.

Live path of the reference (the recurrence is dead code):
    x_proj = x @ W_proj + b_proj            # [B,L,3D]
    gate, _, ogate = split(x_proj, 3)       # middle third is DEAD
    out = (gate) * sigmoid(ogate)           # [B,L,D]
    out = out @ W_out + b_out               # [B,L,D]

Strategy:
  - Data-parallel over B*L rows: 16384 rows -> 2048 rows/core, no collectives.
  - Feature-major layout on device: host transposes x shard -> xT [D, rows]
    so every matmul contracts over the SBUF partition dim.
  - g-path and output matmuls in bf16 (PSUM accumulates fp32).
  - o-gate matmul in fp8e4m3 with DoubleRow (2 contraction rows per PE cell,
    ~1.7x bf16 MM rate). The sigmoid damps the fp8 quantization noise ~2.5x,
    so only this matmul tolerates fp8 within the error budget. Weights are
    pre-scaled x16 into fp8 range; the 1/16 folds into the activation scale.
  - Prologue is HBM-bandwidth-bound (~4.5MB needed in the first ~12us), so
    DRAM layouts are per-partition-contiguous (4-8KB DMA lines), transfers
    are chunked in need order, and spread across the gpsimd SWDGE queue
    (~210GB/s) and both HWDGE queues (~65GB/s each). The first row block
    runs all 8 o-gate PSUM groups before the first g-group so the PE only
    needs x8+wo8 (1MB) to start, and wg/wu are chunked by m-half to match
    their first-use times.
"""

import os

import numpy as np
import ml_dtypes

try:
    import concourse.bass as bass
except ImportError:
    import sys

    sys.path.insert(0, "/opt/trn_rl_repo")
    import concourse.bass as bass

import concourse.mybir as mybir
from concourse import bacc
from concourse.tile import TileContext
from concourse.bass_utils import run_bass_kernel_spmd

BF16 = ml_dtypes.bfloat16
F8 = ml_dtypes.float8_e4m3

B, L, D = 4, 4096, 1024
NCORES = 8
ROWS = B * L            # 16384
RPC = ROWS // NCORES    # 2048 rows per core
RB = 512                # moving free-dim per matmul (= one fp32 PSUM bank)
NRB = RPC // RB         # 4 row blocks per core
P = 128                 # SBUF partitions
KT = D // P             # 8 contraction tiles
KK = KT // 2            # 4 DoubleRow contraction pair-tiles
HD = D // 2             # weight m-half chunk
QD = D // 4             # weight m-quarter chunk
WO_SCALE = 16.0         # o-gate weights pre-scaled into fp8 range

_NC = None
LAST_RESULT = None      # BassKernelResults of the most recent run (for test.py)


def _build():
    nc = bacc.Bacc(trn_type="TRN2")
    f32 = mybir.dt.float32
    bf16 = mybir.dt.bfloat16
    fp8 = mybir.dt.float8e4
    DR = mybir.MatmulPerfMode.DoubleRow

    # all layouts per-partition contiguous for large DMA lines
    xb = nc.dram_tensor("xb", [NRB, P, KT, RB], bf16, kind="ExternalInput")
    x8 = nc.dram_tensor("x8", [NRB, P, KK, 2, RB], fp8, kind="ExternalInput")
    wg = nc.dram_tensor("wg", [4, P, KT, QD], bf16, kind="ExternalInput")
    wu = nc.dram_tensor("wu", [2, P, KT, HD], bf16, kind="ExternalInput")
    wo8 = nc.dram_tensor("wo8", [P, KK, 2, D], fp8, kind="ExternalInput")
    bias = nc.dram_tensor("bias", [P, 3, KT], f32, kind="ExternalInput")
    yT = nc.dram_tensor("yT", [D, RPC], bf16, kind="ExternalOutput")

    with TileContext(nc) as tc:
        with (
            tc.tile_pool(name="const", bufs=1) as cpool,
            tc.tile_pool(name="work", bufs=2) as wpool,
            tc.tile_pool(name="outp", bufs=4) as opool,
            tc.tile_pool(name="ps", bufs=2, space="PSUM") as pspool,
        ):
            biasS = cpool.tile([P, 3, KT], f32, tag="bias", name="biasS")
            wo8S = cpool.tile([P, KK, 2, D], fp8, tag="wo8", name="wo8S")
            x8S = cpool.tile([P, NRB, KK, 2, RB], fp8, tag="x8", name="x8S")
            xbS = cpool.tile([P, NRB, KT, RB], bf16, tag="xb", name="xbS")
            wgS = cpool.tile([P, KT, D], bf16, tag="wg", name="wgS")
            wuS = cpool.tile([P, KT, D], bf16, tag="wu", name="wuS")

            # --- Prologue DMAs, in need order per queue. Aggregate HBM read
            # BW in the prologue is ~310GB/s and the SDMA engines round-robin
            # between busy queues (~1/3 share each), so the critical set
            # (wo8 + x8 block 0, 1.5MB) is split across all three queues and
            # everything else follows in phase-A/B/C need order.
            # The SDMA engines give each busy queue an equal HBM share
            # (~100GB/s with three queues active), so inputs are balanced
            # ~3.5MB per queue and strictly need-ordered within each.
            # gpsimd SWDGE: o-gate weights, g-path weights
            nc.gpsimd.dma_start(out=wo8S[:, 0:2], in_=wo8[:, 0:2])
            nc.gpsimd.dma_start(out=wo8S[:, 2:4], in_=wo8[:, 2:4])
            for q in range(4):
                nc.gpsimd.dma_start(out=wgS[:, :, q * QD : (q + 1) * QD], in_=wg[q])
            # sync HWDGE: x8 blocks in phase-A order, xb blocks 1-2;
            # even-n yo output DMAs join this queue in phase C.
            for rb in range(NRB):
                nc.sync.dma_start(out=x8S[:, rb], in_=x8[rb])
            nc.sync.dma_start(out=xbS[:, 1], in_=xb[1])
            nc.sync.dma_start(out=xbS[:, 2], in_=xb[2])
            # scalar HWDGE: bias, xb block 0 halves, wu, xb block 3;
            # odd-n yo output DMAs join this queue in phase C.
            nc.scalar.dma_start(out=biasS, in_=bias[:])
            nc.scalar.dma_start(out=xbS[:, 0, 4:8], in_=xb[0, :, 4:8])
            nc.scalar.dma_start(out=xbS[:, 0, 0:4], in_=xb[0, :, 0:4])
            nc.scalar.dma_start(out=wuS[:, :, 0:HD], in_=wu[0])
            nc.scalar.dma_start(out=xbS[:, 3], in_=xb[3])
            nc.scalar.dma_start(out=wuS[:, :, HD:D], in_=wu[1])

            # Warm-up: HAM ungates the PE clock after ~3.4us of sustained
            # activity. Spin until the critical DMA set (wo8 + first x8
            # blocks, ~19us in) has fully landed, so the real matmul stream
            # starts warm and runs dense -- a ragged early start costs more
            # in HAM re-throttle (cold runs at half rate) than it gains.
            wz = cpool.tile([P, RB], bf16, tag="wz", name="wz")
            nc.vector.memset(wz, 0.0)
            spin = pspool.tile([P, RB], f32, tag="ps", name="spin", bufs=8)
            for _ in range(40):
                nc.tensor.matmul(spin, lhsT=wz[:, :P], rhs=wz, start=True, stop=True)

            # ---- phase A: o-gate for all row blocks (needs only wo8 + x8)
            sig_t = {}
            for rb in range(NRB):
                for m in range(KT):
                    msl = slice(m * P, (m + 1) * P)
                    po = pspool.tile([P, RB], f32, tag="ps", name=f"po{rb}_{m}", bufs=8)
                    for kk in range(KK):
                        nc.tensor.matmul(
                            po,
                            lhsT=wo8S[:, kk, :, msl],
                            rhs=x8S[:, rb, kk],
                            start=(kk == 0),
                            stop=(kk == KK - 1),
                            perf_mode=DR,
                        )
                    sig = opool.tile(
                        [P, RB], bf16, tag=f"sig{rb}_{m}", name=f"sig{rb}_{m}",
                        bufs=1,
                    )
                    nc.scalar.activation(
                        out=sig, in_=po,
                        func=mybir.ActivationFunctionType.Sigmoid,
                        bias=biasS[:, 1, m : m + 1], scale=1.0 / WO_SCALE,
                    )
                    sig_t[rb, m] = sig

            # ---- phase B: g-path for all row blocks: gT = (hT+bg) * sig
            gS = {}
            for rb in range(NRB):
                for m in range(KT):
                    msl = slice(m * P, (m + 1) * P)
                    ph = pspool.tile([P, RB], f32, tag="ps", name=f"ph{rb}_{m}", bufs=8)
                    for k in range(KT):
                        nc.tensor.matmul(
                            ph, lhsT=wgS[:, k, msl], rhs=xbS[:, rb, k],
                            start=(k == 0), stop=(k == KT - 1),
                        )
                    g = wpool.tile(
                        [P, RB], bf16, tag=f"g{rb}_{m}", name=f"g{rb}_{m}", bufs=1
                    )
                    nc.vector.scalar_tensor_tensor(
                        out=g, in0=ph, scalar=biasS[:, 0, m : m + 1],
                        in1=sig_t[rb, m],
                        op0=mybir.AluOpType.add, op1=mybir.AluOpType.mult,
                    )
                    gS[rb, m] = g

            # ---- phase C: layer 2 for all row blocks (+ b_out)
            for rb in range(NRB):
                cols = slice(rb * RB, (rb + 1) * RB)
                for n in range(KT):
                    nsl = slice(n * P, (n + 1) * P)
                    py = pspool.tile([P, RB], f32, tag="ps", name=f"py{rb}_{n}", bufs=8)
                    for m in range(KT):
                        nc.tensor.matmul(
                            py, lhsT=wuS[:, m, nsl], rhs=gS[rb, m],
                            start=(m == 0), stop=(m == KT - 1),
                        )
                    yo = opool.tile([P, RB], bf16, tag="yo", name=f"yo{rb}_{n}")
                    # bias-add on DVE, keeping ScalarE free for sigmoids
                    nc.vector.tensor_scalar_add(yo, py, biasS[:, 2, n : n + 1])
                    # outputs split across both HWDGE queues (idle by phase C)
                    # so the drain keeps ahead of the production rate; the
                    # final tiles go out as parallel half-width transfers to
                    # shorten the post-compute drain tail
                    if rb == NRB - 1 and n >= KT - 2:
                        c0 = rb * RB
                        nc.sync.dma_start(
                            out=yT[nsl, c0 : c0 + RB // 2], in_=yo[:, 0 : RB // 2]
                        )
                        nc.scalar.dma_start(
                            out=yT[nsl, c0 + RB // 2 : c0 + RB], in_=yo[:, RB // 2 :]
                        )
                    elif n % 2 == 0:
                        nc.sync.dma_start(out=yT[nsl, cols], in_=yo)
                    else:
                        nc.scalar.dma_start(out=yT[nsl, cols], in_=yo)
    nc.finalize()
    return nc


def kernel(x, W_proj, b_proj, W_out, b_out, layer_idx=0, num_layers=12):
    global _NC, LAST_RESULT
    x = np.asarray(x, dtype=np.float32)
    W_proj = np.asarray(W_proj, dtype=np.float32)
    b_proj = np.asarray(b_proj, dtype=np.float32)
    W_out = np.asarray(W_out, dtype=np.float32)
    b_out = np.asarray(b_out, dtype=np.float32)

    wg_f = W_proj[:, :D]
    wo_f = W_proj[:, 2 * D : 3 * D]
    # [4, P, KT, QD]: m-quarter-major, per-partition contiguous
    wg_h = np.ascontiguousarray(
        wg_f.reshape(KT, P, 4, QD).transpose(2, 1, 0, 3)
    ).astype(BF16)
    wu_h = np.ascontiguousarray(
        W_out.reshape(KT, P, 2, HD).transpose(2, 1, 0, 3)
    ).astype(BF16)
    # [P, KK, 2, D]
    wo8_h = np.ascontiguousarray(
        (wo_f * WO_SCALE).reshape(KK, 2, P, D).transpose(2, 0, 1, 3)
    ).astype(F8)
    bias_h = np.ascontiguousarray(
        np.stack(
            [
                b_proj[:D].reshape(KT, P).T,
                b_proj[2 * D : 3 * D].reshape(KT, P).T,
                b_out.reshape(KT, P).T,
            ],
            axis=1,
        )
    )  # [P, 3, KT]

    xf = x.reshape(ROWS, D)
    in_maps = []
    for c in range(NCORES):
        xT = np.ascontiguousarray(xf[c * RPC : (c + 1) * RPC, :].T)  # [D, RPC]
        # [NRB, P, KT, RB]
        xb_h = np.ascontiguousarray(
            xT.reshape(KT, P, NRB, RB).transpose(2, 1, 0, 3)
        ).astype(BF16)
        # [NRB, P, KK, 2, RB]: block-major fp8 pairs for DoubleRow
        x8_h = np.ascontiguousarray(
            xT.reshape(KK, 2, P, NRB, RB).transpose(3, 2, 0, 1, 4)
        ).astype(F8)
        in_maps.append(
            {"xb": xb_h, "x8": x8_h, "wg": wg_h, "wu": wu_h,
             "wo8": wo8_h, "bias": bias_h}
        )

    if _NC is None:
        _NC = _build()

    trace = os.environ.get("HGRN_TRACE", "0") == "1"
    LAST_RESULT = run_bass_kernel_spmd(
        _NC, in_maps, core_ids=list(range(NCORES)), trace=trace,
        tmpdir=os.environ.get("HGRN_TMPDIR"),
    )
    y = np.empty((ROWS, D), dtype=np.float32)
    for c in range(NCORES):
        y[c * RPC : (c + 1) * RPC, :] = np.asarray(
            LAST_RESULT.results[c]["yT"], dtype=np.float32
        ).T
    return y.reshape(B, L, D)
